# revision 2
# baseline (speedup 1.0000x reference)
"""Trainium2 Bass kernel for nn_MixtureOfExperts_77455440216219.

Mixture of 16 expert LSTMs (H=256) over an unbatched sequence of length
4096 (torch LSTM semantics), with dense-then-masked top-2 gating and a
per-expert output projection.

Strategy (expert-parallel over 8 NeuronCores, 2 experts per core):
  Phase A: xg = x @ W_ih^T + (b_ih + b_hh)  -- dense PE matmuls, result
           kept resident in SBUF as fp16 [128, 16, 4096].
  Phase B: the 4096-step LSTM scan.  Per step: 32 weight-stationary
           [128,128] bf16 matmuls (N=1) accumulate the gate pre-activations
           [128, 16 cols] in PSUM; sigmoid/tanh + cell update on ACT/DVE;
           h_t written to an SBUF history buffer (bf16) that feeds the next
           step's matmuls.
  Phase C: out_partial[t, :] = sum_e gated[t,e] * (W_lin[e] @ h[t,e]) via
           PE matmuls over 128-step chunks (lhsT = h history).
  Host: gating (softmax + top-2 mask, replicated math, <0.1% of FLOPs),
        the b_lin bias term, and the final sum over the 8 expert shards.

Gate column order per expert inside the 16-wide gate tile:
  [i0, i1, f0, f1, o0, o1, g0, g1]  (chunks of 128 of the 4H=1024 gate
  vector; i/f/o cols 0..5 get sigmoid in one op, g cols 6..7 get tanh).
"""

import os
import sys

for _p in ("/opt/trn_rl_repo", "/root/.axon_site/_ro/trn_rl_repo"):
    if os.path.isdir(_p) and _p not in sys.path:
        sys.path.insert(0, _p)

import numpy as np
from ml_dtypes import bfloat16 as np_bf16

np_f16 = np.float16

B, D, H, OUT, E, K_TOP = 4096, 128, 256, 16, 16, 2
NCORES = 8
E_LOC = E // NCORES          # 2 experts per core
H4 = 4 * H                   # 1024
KCH = H // 128               # 2 contraction chunks of h
MCH = H4 // 128              # 8 gate chunks per expert
NG = E_LOC * MCH             # 16 gate columns per core
T = B                        # 4096 sequential steps

U = 16                       # scan steps unrolled per For_i iteration

# gate-chunk (0..7 over [i,i,f,f,g,g,o,o]) -> column position
# order [i0,i1,f0,f1,o0,o1,g0,g1]
_COLPOS = {0: 0, 1: 1, 2: 2, 3: 3, 4: 6, 5: 7, 6: 4, 7: 5}
_INV_COLPOS = {v: k for k, v in _COLPOS.items()}

LAST_EXEC_NS = None
LAST_RESULTS = None


def _build_program(t_steps=T, u_unroll=U, n_devices=NCORES):
    import concourse.bacc as bacc
    import concourse.mybir as mybir
    from concourse import bass
    from concourse.tile import TileContext

    f32 = mybir.dt.float32
    f16 = mybir.dt.float16
    bf16 = mybir.dt.bfloat16
    Act = mybir.ActivationFunctionType
    Alu = mybir.AluOpType
    ds = bass.ds

    TT = t_steps
    n_tchunk_a = TT // 512 if TT >= 512 else 1
    tca = min(512, TT)              # phase A time-chunk
    n_tchunk_c = (TT + 127) // 128  # phase C time-chunks

    nc = bacc.Bacc("TRN2", target_bir_lowering=False, debug=False,
                   num_devices=n_devices)

    xt_d = nc.dram_tensor("xt", [128, TT], bf16, kind="ExternalInput")
    wih_d = nc.dram_tensor("wih", [128, NG * 128], bf16, kind="ExternalInput")
    whh_d = nc.dram_tensor("whh", [128, E_LOC * KCH * MCH * 128], bf16,
                           kind="ExternalInput")
    bsum_d = nc.dram_tensor("bsum", [128, NG], f32, kind="ExternalInput")
    wlin_d = nc.dram_tensor("wlin", [128, E_LOC * KCH * OUT], bf16,
                            kind="ExternalInput")
    gated_d = nc.dram_tensor("gated", [128, n_tchunk_c * E_LOC], f32,
                             kind="ExternalInput")
    out_d = nc.dram_tensor("out", [TT, OUT], f32, kind="ExternalOutput")

    with TileContext(nc) as tc:
        with tc.tile_pool(name="persist", bufs=1) as pp:
            xt_sb = pp.tile([128, TT], bf16)
            wih_sb = pp.tile([128, NG * 128], bf16)
            whh_sb = pp.tile([128, E_LOC * KCH * MCH * 128], bf16)
            bsum_sb = pp.tile([128, NG], f32)
            wlin_sb = pp.tile([128, E_LOC * KCH * OUT], bf16)
            gated_sb = pp.tile([128, n_tchunk_c * E_LOC], f32)
            xg_sb = pp.tile([128, NG, TT], f16)
            hh_sb = pp.tile([128, E_LOC * KCH, TT + 1], bf16)
            c_sb = pp.tile([128, E_LOC, KCH], f32)

            nc.sync.dma_start(xt_sb[:], xt_d[:])
            nc.sync.dma_start(wih_sb[:], wih_d[:])
            nc.sync.dma_start(whh_sb[:], whh_d[:])
            nc.sync.dma_start(bsum_sb[:], bsum_d[:])
            nc.sync.dma_start(wlin_sb[:], wlin_d[:])
            nc.sync.dma_start(gated_sb[:], gated_d[:])

            nc.vector.memset(hh_sb[:, :, 0], 0.0)
            nc.vector.memset(c_sb[:], 0.0)

            # ---- Phase A: xg = W_ih @ x^T + b, laid out [128, col, t] ----
            with (
                tc.tile_pool(name="psA", bufs=2, space="PSUM") as psA,
                tc.tile_pool(name="wkA", bufs=2) as _wkA,
            ):
                for tch in range(n_tchunk_a):
                    t0 = tch * tca
                    for col in range(NG):
                        ps = psA.tile([128, tca], f32)
                        nc.tensor.matmul(
                            ps[:],
                            lhsT=wih_sb[:, col * 128:(col + 1) * 128],
                            rhs=xt_sb[:, t0:t0 + tca],
                            start=True, stop=True,
                        )
                        nc.vector.tensor_scalar_add(
                            xg_sb[:, col, t0:t0 + tca], ps[:],
                            bsum_sb[:, col:col + 1],
                        )

            # ---- Phase B: the scan ----
            with (
                tc.tile_pool(name="psB", bufs=4, space="PSUM") as psB,
                tc.tile_pool(name="wkB", bufs=4) as wkB,
            ):
                def scan_step(t_sym, t_next_sym):
                    g_ps = psB.tile([128, NG], f32, tag="g_ps")
                    for e in range(E_LOC):
                        for cp in range(MCH):
                            col = e * MCH + cp
                            gc = _INV_COLPOS[cp]
                            for k in range(KCH):
                                w0 = (((e * KCH + k) * MCH + gc) * 128)
                                nc.tensor.matmul(
                                    g_ps[:, col:col + 1],
                                    lhsT=whh_sb[:, w0:w0 + 128],
                                    rhs=hh_sb[:, e * KCH + k, t_sym],
                                    start=(k == 0), stop=(k == KCH - 1),
                                )
                    sadd = wkB.tile([128, E_LOC, MCH], f32, tag="sadd")
                    nc.vector.tensor_tensor(
                        sadd[:], g_ps[:], xg_sb[:, :, t_sym], Alu.add)
                    sg = wkB.tile([128, E_LOC, 6], f32, tag="sg")
                    nc.scalar.activation(sg[:], sadd[:, :, 0:6], Act.Sigmoid)
                    tg = wkB.tile([128, E_LOC, KCH], f32, tag="tg")
                    nc.scalar.activation(tg[:], sadd[:, :, 6:8], Act.Tanh)
                    t2 = wkB.tile([128, E_LOC, KCH], f32, tag="t2")
                    nc.vector.tensor_tensor(
                        t2[:], sg[:, :, 0:2], tg[:], Alu.mult)
                    nc.vector.tensor_tensor(
                        c_sb[:], sg[:, :, 2:4], c_sb[:], Alu.mult)
                    nc.vector.tensor_tensor(
                        c_sb[:], c_sb[:], t2[:], Alu.add)
                    tcb = wkB.tile([128, E_LOC, KCH], f32, tag="tcb")
                    nc.scalar.activation(tcb[:], c_sb[:], Act.Tanh)
                    nc.vector.tensor_tensor(
                        hh_sb[:, :, t_next_sym], sg[:, :, 4:6], tcb[:],
                        Alu.mult)

                n_iter = t_steps // u_unroll
                with tc.For_i(0, t_steps, u_unroll) as i0:
                    for u in range(u_unroll):
                        scan_step(ds(i0 + u, 1), ds(i0 + u + 1, 1))

            # ---- Phase C: projection + gated combine ----
            with (
                tc.tile_pool(name="psC", bufs=4, space="PSUM") as psC,
                tc.tile_pool(name="wkC", bufs=4) as wkC,
            ):
                for tch in range(n_tchunk_c):
                    t0 = tch * 128
                    tlen = min(128, TT - t0)
                    acc = wkC.tile([128, OUT], f32, tag="acc")
                    for e in range(E_LOC):
                        ps = psC.tile([128, OUT], f32, tag="ps_c")
                        for k in range(KCH):
                            nc.tensor.matmul(
                                ps[:tlen],
                                lhsT=hh_sb[:, e * KCH + k,
                                           1 + t0:1 + t0 + tlen],
                                rhs=wlin_sb[:, (e * KCH + k) * OUT:
                                            (e * KCH + k + 1) * OUT],
                                start=(k == 0), stop=(k == KCH - 1),
                            )
                        gcol = gated_sb[:, tch * E_LOC + e:
                                        tch * E_LOC + e + 1]
                        if e == 0:
                            nc.vector.tensor_scalar_mul(
                                acc[:tlen], ps[:tlen], gcol[:tlen])
                        else:
                            nc.vector.scalar_tensor_tensor(
                                acc[:tlen], ps[:tlen], gcol[:tlen],
                                acc[:tlen], Alu.mult, Alu.add)
                    nc.sync.dma_start(out_d[t0:t0 + tlen, :], acc[:tlen])

    nc.compile()
    return nc


_PROGRAM_CACHE = {}


def _get_program(t_steps=T, u_unroll=U, n_devices=NCORES):
    key = (t_steps, u_unroll, n_devices)
    if key not in _PROGRAM_CACHE:
        _PROGRAM_CACHE[key] = _build_program(t_steps, u_unroll, n_devices)
    return _PROGRAM_CACHE[key]


def _host_gating(x, Wg, bg):
    """softmax over experts + dense top-2 mask, float32, matching jax."""
    logits = x.astype(np.float32) @ Wg.astype(np.float32).T + bg
    logits -= logits.max(axis=1, keepdims=True)
    ex = np.exp(logits)
    scores = ex / ex.sum(axis=1, keepdims=True)
    second = np.sort(scores, axis=1)[:, -K_TOP][:, None]
    mask = (scores >= second).astype(np.float32)
    return scores * mask


def _prep_core_inputs(core, x, W_ih, W_hh, b_ih, b_hh, W_lin, gated, t_steps):
    e0 = core * E_LOC
    n_tchunk_c = (t_steps + 127) // 128

    xt = np.ascontiguousarray(x[:t_steps].T).astype(np_bf16)

    wih = np.empty((128, NG * 128), np.float32)
    bsum = np.empty((128, NG), np.float32)
    bs = b_ih + b_hh
    for e in range(E_LOC):
        for cp in range(MCH):
            gc = _INV_COLPOS[cp]
            col = e * MCH + cp
            wih[:, col * 128:(col + 1) * 128] = W_ih[e0 + e][gc * 128:(gc + 1) * 128, :].T
            bsum[:, col] = bs[e0 + e][gc * 128:(gc + 1) * 128]

    whh = np.empty((128, E_LOC * KCH * MCH * 128), np.float32)
    for e in range(E_LOC):
        for k in range(KCH):
            for gc in range(MCH):
                w0 = ((e * KCH + k) * MCH + gc) * 128
                whh[:, w0:w0 + 128] = W_hh[e0 + e][gc * 128:(gc + 1) * 128,
                                                   k * 128:(k + 1) * 128].T

    wlin = np.empty((128, E_LOC * KCH * OUT), np.float32)
    for e in range(E_LOC):
        for k in range(KCH):
            wlin[:, (e * KCH + k) * OUT:(e * KCH + k + 1) * OUT] = \
                W_lin[e0 + e][:, k * 128:(k + 1) * 128].T

    gt = np.zeros((128, n_tchunk_c * E_LOC), np.float32)
    for tch in range(n_tchunk_c):
        t0 = tch * 128
        tlen = min(128, t_steps - t0)
        for e in range(E_LOC):
            gt[:tlen, tch * E_LOC + e] = gated[t0:t0 + tlen, e0 + e]

    return {
        "xt": xt,
        "wih": wih.astype(np_bf16),
        "whh": whh.astype(np_bf16),
        "bsum": bsum,
        "wlin": wlin.astype(np_bf16),
        "gated": gt,
    }


def kernel(x, Wg, bg, W_ih, W_hh, b_ih, b_hh, W_lin, b_lin,
           t_steps=T, trace=False):
    global LAST_EXEC_NS, LAST_RESULTS
    from concourse.bass_utils import run_bass_kernel_spmd

    x = np.asarray(x, np.float32)
    gated = _host_gating(np.asarray(x[:t_steps]), np.asarray(Wg, np.float32),
                         np.asarray(bg, np.float32))

    nc = _get_program(t_steps=t_steps)
    in_maps = [
        _prep_core_inputs(c, x, np.asarray(W_ih, np.float32),
                          np.asarray(W_hh, np.float32),
                          np.asarray(b_ih, np.float32),
                          np.asarray(b_hh, np.float32),
                          np.asarray(W_lin, np.float32), gated, t_steps)
        for c in range(NCORES)
    ]
    res = run_bass_kernel_spmd(nc, in_maps, list(range(NCORES)), trace=trace)
    LAST_EXEC_NS = res.exec_time_ns
    LAST_RESULTS = res

    out = np.zeros((t_steps, OUT), np.float32)
    for c in range(NCORES):
        out += res.results[c]["out"]
    out += gated @ np.asarray(b_lin, np.float32)
    return out


# revision 7
# speedup vs baseline: 1.0342x; 1.0342x over previous
"""Trainium2 Bass kernel for nn_MixtureOfExperts_77455440216219.

Mixture of 16 expert LSTMs (H=256) over an unbatched sequence of length
4096 (torch LSTM semantics), with dense-then-masked top-2 gating and a
per-expert output projection.

Strategy (expert-parallel over 8 NeuronCores, 2 experts per core):
  Phase A: xg = x @ W_ih^T + (b_ih + b_hh)  -- dense PE matmuls, result
           kept resident in SBUF as fp16 [128, E, half, pos, 4096].
  Phase B: the 4096-step LSTM scan.  Per step: 32 weight-stationary
           [128,128] bf16 matmuls (N=1), split into two PSUM tiles by
           h-chunk ("half") so the sigmoid/tanh/cell-update chain for
           half 0 runs on ACT/DVE while the PE is still doing half 1's
           matmuls (and vice versa across the step boundary).
  Phase C: out_partial[t, :] = sum_e gated[t,e] * (W_lin[e] @ h[t,e]) via
           PE matmuls over 128-step chunks (lhsT = h history).
  Host: gating (softmax + top-2 mask, replicated math, <0.1% of FLOPs),
        the b_lin bias term, and the final sum over the 8 expert shards.

Gate column order per expert half: [i, f, o, g].  The g (cell-candidate)
pre-activations are pre-scaled by 2 on the host so that
tanh(x) = 2*sigmoid(2x) - 1 lets one sigmoid op cover all four columns.
"""

import os
import sys

for _p in ("/opt/trn_rl_repo", "/root/.axon_site/_ro/trn_rl_repo"):
    if os.path.isdir(_p) and _p not in sys.path:
        sys.path.insert(0, _p)

import numpy as np
from ml_dtypes import bfloat16 as np_bf16

B, D, H, OUT, E, K_TOP = 4096, 128, 256, 16, 16, 2
NCORES = 8
E_LOC = E // NCORES          # 2 experts per core
H4 = 4 * H                   # 1024
KCH = H // 128               # 2 contraction chunks of h ("halves")
MCH = H4 // 128              # 8 gate chunks per expert
NG = E_LOC * MCH             # 16 gate columns per core
T = B                        # 4096 sequential steps

U = 16                       # scan steps unrolled per For_i iteration

# gate-chunk gc (0..7 over [i,i,f,f,g,g,o,o]) -> (half, pos) with
# pos order [i, f, o, g]
_GT2POS = {0: 0, 1: 1, 2: 3, 3: 2}          # gatetype i,f,g,o -> pos


def _gc_to_col(gc):
    half = gc & 1
    pos = _GT2POS[gc >> 1]
    return half * 4 + pos


_COL2GC = {_gc_to_col(gc): gc for gc in range(MCH)}

LAST_EXEC_NS = None
LAST_RESULTS = None


def _build_program(t_steps=T, u_unroll=U, n_devices=NCORES):
    import concourse.bacc as bacc
    import concourse.mybir as mybir
    from concourse import bass
    from concourse.tile import TileContext

    f32 = mybir.dt.float32
    f16 = mybir.dt.float16
    bf16 = mybir.dt.bfloat16
    Act = mybir.ActivationFunctionType
    Alu = mybir.AluOpType
    ds = bass.ds

    TT = t_steps
    n_tchunk_a = TT // 512 if TT >= 512 else 1
    tca = min(512, TT)              # phase A time-chunk
    n_tchunk_c = (TT + 127) // 128  # phase C time-chunks

    nc = bacc.Bacc("TRN2", target_bir_lowering=False, debug=False,
                   num_devices=n_devices)

    xt_d = nc.dram_tensor("xt", [128, TT], bf16, kind="ExternalInput")
    wih_d = nc.dram_tensor("wih", [128, NG * 128], bf16, kind="ExternalInput")
    whh_d = nc.dram_tensor("whh", [128, E_LOC * KCH * MCH * 128], bf16,
                           kind="ExternalInput")
    bsum_d = nc.dram_tensor("bsum", [128, NG], f32, kind="ExternalInput")
    wlin_d = nc.dram_tensor("wlin", [128, E_LOC * KCH * OUT], bf16,
                            kind="ExternalInput")
    gated_d = nc.dram_tensor("gated", [128, n_tchunk_c * E_LOC], f32,
                             kind="ExternalInput")
    out_d = nc.dram_tensor("out", [TT, OUT], f32, kind="ExternalOutput")

    with TileContext(nc) as tc:
        with tc.tile_pool(name="persist", bufs=1) as pp:
            xt_sb = pp.tile([128, TT], bf16)
            wih_sb = pp.tile([128, NG * 128], bf16)
            whh_sb = pp.tile([128, E_LOC * KCH * MCH * 128], bf16)
            bsum_sb = pp.tile([128, NG], f32)
            wlin_sb = pp.tile([128, E_LOC * KCH * OUT], bf16)
            gated_sb = pp.tile([128, n_tchunk_c * E_LOC], f32)
            xg_sb = pp.tile([128, KCH, 4, E_LOC, TT], f16)
            hh_sb = pp.tile([128, E_LOC, KCH, TT + 1], bf16)
            c_sb = pp.tile([128, E_LOC, KCH], f32)

            nc.sync.dma_start(xt_sb[:], xt_d[:])
            nc.sync.dma_start(wih_sb[:], wih_d[:])
            nc.sync.dma_start(whh_sb[:], whh_d[:])
            nc.sync.dma_start(bsum_sb[:], bsum_d[:])
            nc.sync.dma_start(wlin_sb[:], wlin_d[:])
            nc.sync.dma_start(gated_sb[:], gated_d[:])

            nc.vector.memset(hh_sb[:, :, :, 0], 0.0)
            nc.vector.memset(c_sb[:], 0.0)

            # ---- Phase A: xg = W_ih @ x^T + b ----
            with tc.tile_pool(name="psA", bufs=2, space="PSUM") as psA:
                for tch in range(n_tchunk_a):
                    t0 = tch * tca
                    for e in range(E_LOC):
                        for col in range(MCH):
                            gc = _COL2GC[col]
                            half, pos = col // 4, col % 4
                            wcol = e * MCH + col
                            ps = psA.tile([128, tca], f32, tag="ps_a")
                            nc.tensor.matmul(
                                ps[:],
                                lhsT=wih_sb[:, wcol * 128:(wcol + 1) * 128],
                                rhs=xt_sb[:, t0:t0 + tca],
                                start=True, stop=True,
                            )
                            nc.vector.tensor_scalar_add(
                                xg_sb[:, half, pos, e, t0:t0 + tca], ps[:],
                                bsum_sb[:, wcol:wcol + 1],
                            )

            # ---- Phase B: the scan ----
            with (
                tc.tile_pool(name="psB", bufs=2, space="PSUM") as psB,
                tc.tile_pool(name="wkB", bufs=4) as wkB,
            ):
                def scan_step(t_sym, t_next_sym):
                    # G[h][k]: partial gate pre-activations for half h from
                    # h-chunk k.  Single-matmul PSUM groups; emission order
                    # (k0: h0,h1) then (k1: h0,h1) so that step t+1's k0
                    # matmuls only need half 0's chain to have finished.
                    g_part = [[None, None], [None, None]]
                    for k in range(KCH):
                        for h in range(KCH):
                            G = psB.tile([128, 4, E_LOC], f32,
                                         tag=f"g{h}{k}")
                            g_part[h][k] = G
                            for e in range(E_LOC):
                                for pos in range(4):
                                    gc = _COL2GC[h * 4 + pos]
                                    w0 = ((e * KCH + k) * MCH + gc) * 128
                                    nc.tensor.matmul(
                                        G[:, pos, e:e + 1],
                                        lhsT=whh_sb[:, w0:w0 + 128],
                                        rhs=hh_sb[:, e, k, t_sym],
                                        start=True, stop=True,
                                    )
                    for h in range(KCH):
                        # only one PSUM operand allowed per DVE instruction
                        s1 = wkB.tile([128, 4, E_LOC], f32, tag=f"s1{h}")
                        nc.vector.tensor_tensor(
                            s1[:], g_part[h][0][:], xg_sb[:, h, :, :, t_sym],
                            Alu.add)
                        sadd = wkB.tile([128, 4, E_LOC], f32, tag=f"sadd{h}")
                        nc.vector.tensor_tensor(
                            sadd[:], s1[:], g_part[h][1][:], Alu.add)
                        sg = wkB.tile([128, 4, E_LOC], f32, tag=f"sg{h}")
                        nc.scalar.activation(sg[:], sadd[:], Act.Sigmoid)
                        m = wkB.tile([128, E_LOC], f32, tag=f"m{h}")
                        nc.vector.tensor_tensor(
                            m[:], sg[:, 0, :], sg[:, 3, :], Alu.mult)
                        t2 = wkB.tile([128, E_LOC], f32, tag=f"t2{h}")
                        nc.vector.scalar_tensor_tensor(
                            t2[:], m[:], 2.0, sg[:, 0, :],
                            Alu.mult, Alu.subtract)
                        ch = c_sb[:, :, h:h + 1]
                        nc.vector.tensor_tensor(ch, sg[:, 1, :], ch,
                                                Alu.mult)
                        nc.vector.tensor_tensor(ch, ch, t2[:], Alu.add)
                        tcb = wkB.tile([128, E_LOC], f32, tag=f"tcb{h}")
                        nc.scalar.activation(tcb[:], ch, Act.Tanh)
                        nc.vector.tensor_tensor(
                            hh_sb[:, :, h, t_next_sym], sg[:, 2, :],
                            tcb[:], Alu.mult)

                with tc.For_i(0, t_steps, u_unroll) as i0:
                    for u in range(u_unroll):
                        scan_step(ds(i0 + u, 1), ds(i0 + u + 1, 1))

            # ---- Phase C: projection + gated combine ----
            with (
                tc.tile_pool(name="psC", bufs=4, space="PSUM") as psC,
                tc.tile_pool(name="wkC", bufs=4) as wkC,
            ):
                for tch in range(n_tchunk_c):
                    t0 = tch * 128
                    tlen = min(128, TT - t0)
                    acc = wkC.tile([128, OUT], f32, tag="acc")
                    for e in range(E_LOC):
                        ps = psC.tile([128, OUT], f32, tag="ps_c")
                        for k in range(KCH):
                            nc.tensor.matmul(
                                ps[:tlen],
                                lhsT=hh_sb[:, e, k, 1 + t0:1 + t0 + tlen],
                                rhs=wlin_sb[:, (e * KCH + k) * OUT:
                                            (e * KCH + k + 1) * OUT],
                                start=(k == 0), stop=(k == KCH - 1),
                            )
                        gcol = gated_sb[:, tch * E_LOC + e:
                                        tch * E_LOC + e + 1]
                        if e == 0:
                            nc.vector.tensor_scalar_mul(
                                acc[:tlen], ps[:tlen], gcol[:tlen])
                        else:
                            nc.vector.scalar_tensor_tensor(
                                acc[:tlen], ps[:tlen], gcol[:tlen],
                                acc[:tlen], Alu.mult, Alu.add)
                    nc.sync.dma_start(out_d[t0:t0 + tlen, :], acc[:tlen])

    nc.compile()
    return nc


_PROGRAM_CACHE = {}


def _get_program(t_steps=T, u_unroll=U, n_devices=NCORES):
    key = (t_steps, u_unroll, n_devices)
    if key not in _PROGRAM_CACHE:
        _PROGRAM_CACHE[key] = _build_program(t_steps, u_unroll, n_devices)
    return _PROGRAM_CACHE[key]


def _host_gating(x, Wg, bg):
    """softmax over experts + dense top-2 mask, float32, matching jax."""
    logits = x.astype(np.float32) @ Wg.astype(np.float32).T + bg
    logits -= logits.max(axis=1, keepdims=True)
    ex = np.exp(logits)
    scores = ex / ex.sum(axis=1, keepdims=True)
    second = np.sort(scores, axis=1)[:, -K_TOP][:, None]
    mask = (scores >= second).astype(np.float32)
    return scores * mask


def _prep_core_inputs(core, x, W_ih, W_hh, b_ih, b_hh, W_lin, gated, t_steps):
    e0 = core * E_LOC
    n_tchunk_c = (t_steps + 127) // 128

    xt = np.ascontiguousarray(x[:t_steps].T).astype(np_bf16)

    # pre-scale the g (cell candidate) pre-activations by 2 so the kernel
    # can use tanh(x) = 2*sigmoid(2x) - 1
    gscale = np.ones((MCH, 1), np.float32)
    gscale[4] = 2.0   # gc 4,5 = g chunks
    gscale[5] = 2.0

    wih = np.empty((128, NG * 128), np.float32)
    bsum = np.empty((128, NG), np.float32)
    bs = b_ih + b_hh
    for e in range(E_LOC):
        for col in range(MCH):
            gc = _COL2GC[col]
            wcol = e * MCH + col
            wih[:, wcol * 128:(wcol + 1) * 128] = \
                (W_ih[e0 + e][gc * 128:(gc + 1) * 128, :] * gscale[gc]).T
            bsum[:, wcol] = bs[e0 + e][gc * 128:(gc + 1) * 128] * gscale[gc]

    whh = np.empty((128, E_LOC * KCH * MCH * 128), np.float32)
    for e in range(E_LOC):
        for k in range(KCH):
            for gc in range(MCH):
                w0 = ((e * KCH + k) * MCH + gc) * 128
                whh[:, w0:w0 + 128] = \
                    (W_hh[e0 + e][gc * 128:(gc + 1) * 128,
                                  k * 128:(k + 1) * 128] * gscale[gc]).T

    wlin = np.empty((128, E_LOC * KCH * OUT), np.float32)
    for e in range(E_LOC):
        for k in range(KCH):
            wlin[:, (e * KCH + k) * OUT:(e * KCH + k + 1) * OUT] = \
                W_lin[e0 + e][:, k * 128:(k + 1) * 128].T

    gt = np.zeros((128, n_tchunk_c * E_LOC), np.float32)
    for tch in range(n_tchunk_c):
        t0 = tch * 128
        tlen = min(128, t_steps - t0)
        for e in range(E_LOC):
            gt[:tlen, tch * E_LOC + e] = gated[t0:t0 + tlen, e0 + e]

    return {
        "xt": xt,
        "wih": wih.astype(np_bf16),
        "whh": whh.astype(np_bf16),
        "bsum": bsum,
        "wlin": wlin.astype(np_bf16),
        "gated": gt,
    }


def kernel(x, Wg, bg, W_ih, W_hh, b_ih, b_hh, W_lin, b_lin,
           t_steps=T, trace=False):
    global LAST_EXEC_NS, LAST_RESULTS
    from concourse.bass_utils import run_bass_kernel_spmd

    x = np.asarray(x, np.float32)
    gated = _host_gating(np.asarray(x[:t_steps]), np.asarray(Wg, np.float32),
                         np.asarray(bg, np.float32))

    nc = _get_program(t_steps=t_steps)
    in_maps = [
        _prep_core_inputs(c, x, np.asarray(W_ih, np.float32),
                          np.asarray(W_hh, np.float32),
                          np.asarray(b_ih, np.float32),
                          np.asarray(b_hh, np.float32),
                          np.asarray(W_lin, np.float32), gated, t_steps)
        for c in range(NCORES)
    ]
    res = run_bass_kernel_spmd(nc, in_maps, list(range(NCORES)), trace=trace)
    LAST_EXEC_NS = res.exec_time_ns
    LAST_RESULTS = res

    out = np.zeros((t_steps, OUT), np.float32)
    for c in range(NCORES):
        out += res.results[c]["out"]
    out += gated @ np.asarray(b_lin, np.float32)
    return out


# revision 10
# speedup vs baseline: 2.0764x; 2.0077x over previous
"""Trainium2 Bass kernel for nn_MixtureOfExperts_77455440216219.

Mixture of 16 expert LSTMs (H=256) over an unbatched sequence of length
4096 (torch LSTM semantics), with dense-then-masked top-2 gating and a
per-expert output projection.

Strategy (expert-parallel over 8 NeuronCores, 2 experts per core):
  Phase A: xg = x @ W_ih^T + (b_ih + b_hh)  -- dense PE matmuls, result
           kept resident in SBUF as fp16 [128, E, half, pos, 4096].
  Phase B: the 4096-step LSTM scan.  Per step: 32 weight-stationary
           [128,128] bf16 matmuls (N=1), split into two PSUM tiles by
           h-chunk ("half") so the sigmoid/tanh/cell-update chain for
           half 0 runs on ACT/DVE while the PE is still doing half 1's
           matmuls (and vice versa across the step boundary).
  Phase C: out_partial[t, :] = sum_e gated[t,e] * (W_lin[e] @ h[t,e]) via
           PE matmuls over 128-step chunks (lhsT = h history).
  Host: gating (softmax + top-2 mask, replicated math, <0.1% of FLOPs),
        the b_lin bias term, and the final sum over the 8 expert shards.

Gate column order per expert half: [i, f, o, g].  The g (cell-candidate)
pre-activations are pre-scaled by 2 on the host so that
tanh(x) = 2*sigmoid(2x) - 1 lets one sigmoid op cover all four columns.
"""

import os
import sys

for _p in ("/opt/trn_rl_repo", "/root/.axon_site/_ro/trn_rl_repo"):
    if os.path.isdir(_p) and _p not in sys.path:
        sys.path.insert(0, _p)

import numpy as np
from ml_dtypes import bfloat16 as np_bf16

B, D, H, OUT, E, K_TOP = 4096, 128, 256, 16, 16, 2
NCORES = 8
E_LOC = E // NCORES          # 2 experts per core
H4 = 4 * H                   # 1024
KCH = H // 128               # 2 contraction chunks of h ("halves")
MCH = H4 // 128              # 8 gate chunks per expert
NG = E_LOC * MCH             # 16 gate columns per core
T = B                        # 4096 sequential steps

U = 16                       # scan steps unrolled per For_i iteration

# gate-chunk gc (0..7 over [i,i,f,f,g,g,o,o]) -> (half, pos) with
# pos order [i, f, o, g]
_GT2POS = {0: 0, 1: 1, 2: 3, 3: 2}          # gatetype i,f,g,o -> pos


def _gc_to_col(gc):
    half = gc & 1
    pos = _GT2POS[gc >> 1]
    return half * 4 + pos


_COL2GC = {_gc_to_col(gc): gc for gc in range(MCH)}

LAST_EXEC_NS = None
LAST_RESULTS = None


def _build_program(t_steps=T, u_unroll=U, n_devices=NCORES):
    import concourse.bacc as bacc
    import concourse.mybir as mybir
    from concourse import bass
    from concourse.tile import TileContext

    f32 = mybir.dt.float32
    f16 = mybir.dt.float16
    bf16 = mybir.dt.bfloat16
    Act = mybir.ActivationFunctionType
    Alu = mybir.AluOpType
    ds = bass.ds

    TT = t_steps
    n_tchunk_a = TT // 512 if TT >= 512 else 1
    tca = min(512, TT)              # phase A time-chunk
    n_tchunk_c = (TT + 127) // 128  # phase C time-chunks

    nc = bacc.Bacc("TRN2", target_bir_lowering=False, debug=False,
                   num_devices=n_devices)

    xt_d = nc.dram_tensor("xt", [128, TT], bf16, kind="ExternalInput")
    wih_d = nc.dram_tensor("wih", [128, NG * 128], bf16, kind="ExternalInput")
    whh_d = nc.dram_tensor("whh", [128, E_LOC * KCH * MCH * 128], bf16,
                           kind="ExternalInput")
    bsum_d = nc.dram_tensor("bsum", [128, NG], f32, kind="ExternalInput")
    wlin_d = nc.dram_tensor("wlin", [128, E_LOC * KCH * OUT], bf16,
                            kind="ExternalInput")
    gated_d = nc.dram_tensor("gated", [128, n_tchunk_c * E_LOC], f32,
                             kind="ExternalInput")
    out_d = nc.dram_tensor("out", [TT, OUT], f32, kind="ExternalOutput")

    with TileContext(nc) as tc:
        with tc.tile_pool(name="persist", bufs=1) as pp:
            xt_sb = pp.tile([128, TT], bf16)
            wih_sb = pp.tile([128, NG * 128], bf16)
            whh_sb = pp.tile([128, E_LOC * KCH * MCH * 128], bf16)
            bsum_sb = pp.tile([128, NG], f32)
            wlin_sb = pp.tile([128, E_LOC * KCH * OUT], bf16)
            gated_sb = pp.tile([128, n_tchunk_c * E_LOC], f32)
            xg_sb = pp.tile([128, KCH, 4, E_LOC, TT], f16)
            hh_sb = pp.tile([128, E_LOC, KCH, TT + 1], bf16)
            c_sb = pp.tile([128, E_LOC, KCH], f32)
            # ping-pong current-h tiles (static APs for the PE rhs)
            hp = [[pp.tile([128, E_LOC], bf16, name=f"hp{_par}{_h}")
                   for _h in range(KCH)] for _par in range(2)]

            nc.sync.dma_start(xt_sb[:], xt_d[:])
            nc.sync.dma_start(wih_sb[:], wih_d[:])
            nc.sync.dma_start(whh_sb[:], whh_d[:])
            nc.sync.dma_start(bsum_sb[:], bsum_d[:])
            nc.sync.dma_start(wlin_sb[:], wlin_d[:])
            nc.sync.dma_start(gated_sb[:], gated_d[:])

            nc.vector.memset(hh_sb[:, :, :, 0], 0.0)
            nc.vector.memset(c_sb[:], 0.0)
            for _par in range(2):
                for _h in range(KCH):
                    nc.vector.memset(hp[_par][_h][:], 0.0)

            # ---- Phase A: xg = W_ih @ x^T + b ----
            with tc.tile_pool(name="psA", bufs=2, space="PSUM") as psA:
                for tch in range(n_tchunk_a):
                    t0 = tch * tca
                    for e in range(E_LOC):
                        for col in range(MCH):
                            gc = _COL2GC[col]
                            half, pos = col // 4, col % 4
                            wcol = e * MCH + col
                            ps = psA.tile([128, tca], f32, tag="ps_a")
                            nc.tensor.matmul(
                                ps[:],
                                lhsT=wih_sb[:, wcol * 128:(wcol + 1) * 128],
                                rhs=xt_sb[:, t0:t0 + tca],
                                start=True, stop=True,
                            )
                            nc.vector.tensor_scalar_add(
                                xg_sb[:, half, pos, e, t0:t0 + tca], ps[:],
                                bsum_sb[:, wcol:wcol + 1],
                            )

            # ---- Phase B: the scan ----
            with (
                tc.tile_pool(name="psB", bufs=2, space="PSUM") as psB,
                tc.tile_pool(name="wkB", bufs=4) as wkB,
            ):
                def scan_step(t_sym, t_next_sym, par):
                    # G[h][k]: partial gate pre-activations for half h from
                    # h-chunk k.  Single-matmul PSUM groups; emission order
                    # (k0: h0,h1) then (k1: h0,h1) so that step t+1's k0
                    # matmuls only need half 0's chain to have finished.
                    g_part = [[None, None], [None, None]]
                    for k in range(KCH):
                        for h in range(KCH):
                            G = psB.tile([128, 4, E_LOC], f32,
                                         tag=f"g{h}{k}")
                            g_part[h][k] = G
                            for e in range(E_LOC):
                                for pos in range(4):
                                    gc = _COL2GC[h * 4 + pos]
                                    w0 = ((e * KCH + k) * MCH + gc) * 128
                                    nc.tensor.matmul(
                                        G[:, pos, e:e + 1],
                                        lhsT=whh_sb[:, w0:w0 + 128],
                                        rhs=hp[1 - par][k][:, e:e + 1],
                                        start=True, stop=True,
                                    )
                    for h in range(KCH):
                        # only one PSUM operand allowed per DVE instruction
                        s1 = wkB.tile([128, 4, E_LOC], f32, tag=f"s1{h}")
                        nc.vector.tensor_tensor(
                            s1[:], g_part[h][0][:], xg_sb[:, h, :, :, t_sym],
                            Alu.add)
                        sadd = wkB.tile([128, 4, E_LOC], f32, tag=f"sadd{h}")
                        nc.vector.tensor_tensor(
                            sadd[:], s1[:], g_part[h][1][:], Alu.add)
                        sg = wkB.tile([128, 4, E_LOC], f32, tag=f"sg{h}")
                        nc.scalar.activation(sg[:], sadd[:], Act.Sigmoid)
                        m = wkB.tile([128, E_LOC], f32, tag=f"m{h}")
                        nc.vector.tensor_tensor(
                            m[:], sg[:, 0, :], sg[:, 3, :], Alu.mult)
                        t2 = wkB.tile([128, E_LOC], f32, tag=f"t2{h}")
                        nc.vector.scalar_tensor_tensor(
                            t2[:], m[:], 2.0, sg[:, 0, :],
                            Alu.mult, Alu.subtract)
                        ch = c_sb[:, :, h:h + 1]
                        nc.vector.tensor_tensor(ch, sg[:, 1, :], ch,
                                                Alu.mult)
                        nc.vector.tensor_tensor(ch, ch, t2[:], Alu.add)
                        tcb = wkB.tile([128, E_LOC], f32, tag=f"tcb{h}")
                        nc.scalar.activation(tcb[:], ch, Act.Tanh)
                        nc.vector.tensor_tensor(
                            hp[par][h][:], sg[:, 2, :], tcb[:], Alu.mult)
                        # history write for phase C, off the critical path
                        nc.gpsimd.tensor_copy(
                            hh_sb[:, :, h, t_next_sym], hp[par][h][:])

                assert u_unroll % 2 == 0
                with tc.For_i(0, t_steps, u_unroll) as i0:
                    for u in range(u_unroll):
                        scan_step(ds(i0 + u, 1), ds(i0 + u + 1, 1), u % 2)

            # ---- Phase C: projection + gated combine ----
            with (
                tc.tile_pool(name="psC", bufs=4, space="PSUM") as psC,
                tc.tile_pool(name="wkC", bufs=4) as wkC,
            ):
                for tch in range(n_tchunk_c):
                    t0 = tch * 128
                    tlen = min(128, TT - t0)
                    acc = wkC.tile([128, OUT], f32, tag="acc")
                    for e in range(E_LOC):
                        ps = psC.tile([128, OUT], f32, tag="ps_c")
                        for k in range(KCH):
                            nc.tensor.matmul(
                                ps[:tlen],
                                lhsT=hh_sb[:, e, k, 1 + t0:1 + t0 + tlen],
                                rhs=wlin_sb[:, (e * KCH + k) * OUT:
                                            (e * KCH + k + 1) * OUT],
                                start=(k == 0), stop=(k == KCH - 1),
                            )
                        gcol = gated_sb[:, tch * E_LOC + e:
                                        tch * E_LOC + e + 1]
                        if e == 0:
                            nc.vector.tensor_scalar_mul(
                                acc[:tlen], ps[:tlen], gcol[:tlen])
                        else:
                            nc.vector.scalar_tensor_tensor(
                                acc[:tlen], ps[:tlen], gcol[:tlen],
                                acc[:tlen], Alu.mult, Alu.add)
                    nc.sync.dma_start(out_d[t0:t0 + tlen, :], acc[:tlen])

    nc.compile()
    return nc


_PROGRAM_CACHE = {}


def _get_program(t_steps=T, u_unroll=U, n_devices=NCORES):
    key = (t_steps, u_unroll, n_devices)
    if key not in _PROGRAM_CACHE:
        _PROGRAM_CACHE[key] = _build_program(t_steps, u_unroll, n_devices)
    return _PROGRAM_CACHE[key]


def _host_gating(x, Wg, bg):
    """softmax over experts + dense top-2 mask, float32, matching jax."""
    logits = x.astype(np.float32) @ Wg.astype(np.float32).T + bg
    logits -= logits.max(axis=1, keepdims=True)
    ex = np.exp(logits)
    scores = ex / ex.sum(axis=1, keepdims=True)
    second = np.sort(scores, axis=1)[:, -K_TOP][:, None]
    mask = (scores >= second).astype(np.float32)
    return scores * mask


def _prep_core_inputs(core, x, W_ih, W_hh, b_ih, b_hh, W_lin, gated, t_steps):
    e0 = core * E_LOC
    n_tchunk_c = (t_steps + 127) // 128

    xt = np.ascontiguousarray(x[:t_steps].T).astype(np_bf16)

    # pre-scale the g (cell candidate) pre-activations by 2 so the kernel
    # can use tanh(x) = 2*sigmoid(2x) - 1
    gscale = np.ones((MCH, 1), np.float32)
    gscale[4] = 2.0   # gc 4,5 = g chunks
    gscale[5] = 2.0

    wih = np.empty((128, NG * 128), np.float32)
    bsum = np.empty((128, NG), np.float32)
    bs = b_ih + b_hh
    for e in range(E_LOC):
        for col in range(MCH):
            gc = _COL2GC[col]
            wcol = e * MCH + col
            wih[:, wcol * 128:(wcol + 1) * 128] = \
                (W_ih[e0 + e][gc * 128:(gc + 1) * 128, :] * gscale[gc]).T
            bsum[:, wcol] = bs[e0 + e][gc * 128:(gc + 1) * 128] * gscale[gc]

    whh = np.empty((128, E_LOC * KCH * MCH * 128), np.float32)
    for e in range(E_LOC):
        for k in range(KCH):
            for gc in range(MCH):
                w0 = ((e * KCH + k) * MCH + gc) * 128
                whh[:, w0:w0 + 128] = \
                    (W_hh[e0 + e][gc * 128:(gc + 1) * 128,
                                  k * 128:(k + 1) * 128] * gscale[gc]).T

    wlin = np.empty((128, E_LOC * KCH * OUT), np.float32)
    for e in range(E_LOC):
        for k in range(KCH):
            wlin[:, (e * KCH + k) * OUT:(e * KCH + k + 1) * OUT] = \
                W_lin[e0 + e][:, k * 128:(k + 1) * 128].T

    gt = np.zeros((128, n_tchunk_c * E_LOC), np.float32)
    for tch in range(n_tchunk_c):
        t0 = tch * 128
        tlen = min(128, t_steps - t0)
        for e in range(E_LOC):
            gt[:tlen, tch * E_LOC + e] = gated[t0:t0 + tlen, e0 + e]

    return {
        "xt": xt,
        "wih": wih.astype(np_bf16),
        "whh": whh.astype(np_bf16),
        "bsum": bsum,
        "wlin": wlin.astype(np_bf16),
        "gated": gt,
    }


def kernel(x, Wg, bg, W_ih, W_hh, b_ih, b_hh, W_lin, b_lin,
           t_steps=T, trace=False):
    global LAST_EXEC_NS, LAST_RESULTS
    from concourse.bass_utils import run_bass_kernel_spmd

    x = np.asarray(x, np.float32)
    gated = _host_gating(np.asarray(x[:t_steps]), np.asarray(Wg, np.float32),
                         np.asarray(bg, np.float32))

    nc = _get_program(t_steps=t_steps)
    in_maps = [
        _prep_core_inputs(c, x, np.asarray(W_ih, np.float32),
                          np.asarray(W_hh, np.float32),
                          np.asarray(b_ih, np.float32),
                          np.asarray(b_hh, np.float32),
                          np.asarray(W_lin, np.float32), gated, t_steps)
        for c in range(NCORES)
    ]
    res = run_bass_kernel_spmd(nc, in_maps, list(range(NCORES)), trace=trace)
    LAST_EXEC_NS = res.exec_time_ns
    LAST_RESULTS = res

    out = np.zeros((t_steps, OUT), np.float32)
    for c in range(NCORES):
        out += res.results[c]["out"]
    out += gated @ np.asarray(b_lin, np.float32)
    return out
